# revision 1
# baseline (speedup 1.0000x reference)
"""Multi-head causal attention (B=4, S=2048, D=1024, H=16) on 8 NeuronCores.

Sharding: core c handles batch b = c//2 and head-group g = c%2 (8 heads).
Each core computes QKV projections for its group, causal attention for its
8 heads, and a partial output projection (row-split Wo).  Host sums the two
partials per batch and adds bo.

On-chip design (per core), all matmuls fp16 with fp32 PSUM accumulation:
  XT [D, S] = X[b].T in 8 chunks [128, S].
  QT/KT pair-tiles [128, S]: partitions 0-63 = head 2c, 64-127 = head 2c+1,
  computed as W.T-chunk (stationary) x XT (moving), bias added during PSUM
  evacuation on ScalarE (per-partition bias AP).
  V per s-chunk [128, 8, 66]: 64 V cols (+bv via broadcast tile) then
  [1,0] (even heads) / [0,1] (odd heads) columns so the PV matmul emits
  rowsum(exp(scores)) at psum row 64 / 65.
  scoresT tiles [sk=128, sq<=512] via two row-tiled K=64 matmuls (head pair
  shares the PE array, separate PSUM banks).  Causal masking: windowed
  matmuls skip fully-masked regions; diagonal 128x128 blocks get -30000
  added via an identity x mask matmul accumulate (keeps exp's deps on PE
  only).  exp on ScalarE (no max subtraction; |scores| <= ~3).
  Normalization: rowsums DMA-scattered across 128 partitions, DVE
  reciprocal, DMA-gathered back to partitions 64-65, broadcast down
  partitions with a K=2 indicator matmul (tile_position row 64), folded
  into OT via one tensor_mul per 512-slice.

Walrus wait-slot legality (1 sem wait per ACT/DVE/DMA instruction): touch
ops pre-observe constant DMAs, reused DVE-written tiles are pre-memset so
the memset absorbs the WAR wait, and the normalization tiles use
one-buffer-per-pair pools so slots are never reused.
"""

import sys

for _p in ("/opt/trn_rl_repo",):
    if _p not in sys.path:
        sys.path.insert(0, _p)

from contextlib import ExitStack

import numpy as np

import concourse.bass as bass
import concourse.mybir as mybir
import concourse.tile as tile
from concourse.bass_utils import run_bass_kernel_spmd

import bass_rust

F16 = mybir.dt.float16
F32 = mybir.dt.float32
AF = mybir.ActivationFunctionType

B, S, D, H = 4, 2048, 1024, 16
HD = D // H  # 64
GH = 8  # heads per group
GW = GH * HD  # 512 columns per group


_SPLITTABLE = {
    "InstMatmult", "InstLdweights", "InstActivation", "InstTensorCopy",
    "InstTensorTensor", "InstTensorScalarPtr", "InstTensorReduce",
    "InstMemset", "InstDMACopy", "InstReciprocal", "InstIota",
    "InstTensorTensorReduce", "InstBNStats", "InstBNStatsAggregate",
    "InstStreamShuffle", "InstNoOp", "InstPool", "InstMax", "InstDrain",
}


def _legalize_waits(nc, max_waits=1):
    """Walrus codegen accepts at most one sync-wait command per engine
    instruction; Tile's wait assigner can emit more.  Split extras onto
    same-engine NoOps inserted immediately before (semantics preserved:
    the engine blocks at the same program point)."""
    ctr = 0
    for fn in nc.m.functions:
        for blk in fn.blocks:
            out = []
            for ins in blk.instructions:
                si = ins.sync_info
                if (
                    si is not None
                    and len(si.on_wait) > max_waits
                    and type(ins).__name__ in _SPLITTABLE
                ):
                    waits = list(si.on_wait)
                    extra, keep = waits[:-max_waits], waits[-max_waits:]
                    for w in extra:
                        nop = mybir.InstNoOp(name=f"waitnop-{ctr}", ins=[], outs=[])
                        ctr += 1
                        nop.engine = ins.engine
                        nop.sync_info = bass_rust.SyncInfo(on_wait=[w], on_update=[])
                        out.append(nop)
                    ins.sync_info = bass_rust.SyncInfo(
                        on_wait=keep, on_update=list(si.on_update)
                    )
                out.append(ins)
            blk.instructions[:] = out
    return ctr


def build_nc(s=S, legalize=True, reps=1):
    ns = s // 512  # 512-wide sq slices per head
    nt = s // 128  # 128-wide s chunks
    nd = D // 128  # contraction chunks for projections
    nb = ns * 512 // 32  # 32-elem blocks per rowsum row (scatter layout)

    nc = bass.Bass("TRN2", target_bir_lowering=False, debug=False)
    xt_d = nc.dram_tensor("xt", [D, s], F16, kind="ExternalInput").ap()
    wq_d = nc.dram_tensor("wq", [D, GW], F16, kind="ExternalInput").ap()
    wk_d = nc.dram_tensor("wk", [D, GW], F16, kind="ExternalInput").ap()
    wv_d = nc.dram_tensor("wv", [D, GW], F16, kind="ExternalInput").ap()
    wo_d = nc.dram_tensor("wo", [GW, D], F16, kind="ExternalInput").ap()
    bqk_d = nc.dram_tensor("bqk", [128, 8], F32, kind="ExternalInput").ap()
    bvb_d = nc.dram_tensor("bvb", [128, GW], F16, kind="ExternalInput").ap()
    mask_d = nc.dram_tensor("mask", [128, 128], F16, kind="ExternalInput").ap()
    out_d = nc.dram_tensor("out", [s, D], F32, kind="ExternalOutput").ap()

    with tile.TileContext(nc) as tc, ExitStack() as ctx:
        pool = lambda name, bufs, **kw: ctx.enter_context(
            tc.tile_pool(name=name, bufs=bufs, **kw)
        )
        const_p = pool("const", 1)
        xt_p = pool("xtp", nd)
        w_p = pool("wp", 1)
        qt_p = pool("qtp", 4)
        kt_p = pool("ktp", 4)
        v_p = pool("vp", nt)
        et_p = pool("etp", 6)
        ot_p = pool("otp", 4)
        rs_p = pool("rsp", 4)
        tmp_p = pool("tmpp", 4)
        ob_p = pool("obp", 4)
        ps_proj = pool("psproj", 2, space="PSUM")
        ps_qk = pool("psqk", 2, space="PSUM")
        ps_pv = pool("pspv", 2, space="PSUM")

        for _rep in range(reps):
            # --- inputs, in order of first use: wv + xt feed the V
            # projection, then bvb, wq/wk, bqk, mask; wo only at the end ---
            wq_sb = w_p.tile([128, nd, GW], F16)
            wk_sb = w_p.tile([128, nd, GW], F16)
            wv_sb = w_p.tile([128, nd, GW], F16)
            wo_sb = w_p.tile([128, 4, D], F16)
            nc.sync.dma_start(out=wv_sb[:], in_=wv_d.rearrange("(d p) n -> p d n", p=128))
            xt_sb = []
            for d in range(nd):
                t = xt_p.tile([128, s], F16, tag="xt", name=f"xtc{d}")
                nc.sync.dma_start(out=t[:], in_=xt_d[d * 128 : (d + 1) * 128, :])
                xt_sb.append(t)
            bvb_sb = const_p.tile([128, GW], F16)
            nc.sync.dma_start(out=bvb_sb[:], in_=bvb_d[:])
            nc.sync.dma_start(out=wq_sb[:], in_=wq_d.rearrange("(d p) n -> p d n", p=128))
            nc.sync.dma_start(out=wk_sb[:], in_=wk_d.rearrange("(d p) n -> p d n", p=128))
            bqk_sb = const_p.tile([128, 8], F32)
            nc.sync.dma_start(out=bqk_sb[:], in_=bqk_d[:])
            mask_sb = const_p.tile([128, 128], F16)
            nc.sync.dma_start(out=mask_sb[:], in_=mask_d[:])
            nc.sync.dma_start(out=wo_sb[:], in_=wo_d.rearrange("(c p) n -> p c n", p=128))

            # touch ops: early ACT-table load + const observations
            scr_a = const_p.tile([128, 1], F32)
            nc.scalar.copy(scr_a[:], bqk_sb[:, 0:1])
            scr_v = const_p.tile([128, 1], F16)
            nc.vector.tensor_copy(scr_v[:], bvb_sb[:, 0:1])
            scr_m = const_p.tile([128, 1], F16)
            nc.vector.tensor_copy(scr_m[:], mask_sb[:, 0:1])

            # --- Q/K projections: QT/KT pair-tiles [128, s] ---
            qt_sb = [qt_p.tile([128, s], F16, tag="qt", name=f"qt{c}") for c in range(4)]
            kt_sb = [kt_p.tile([128, s], F16, tag="kt", name=f"kt{c}") for c in range(4)]
            ot_sb = [ot_p.tile([128, s], F16, tag="ot", name=f"ot{c}") for c in range(4)]
            # --- V projection: per s-chunk [128, 8, 66] with rowsum cols ---
            v_sb = []
            for st in range(nt):
                ps = ps_proj.tile([128, 512], F32, tag="ps", name="ps")
                for d in range(nd):
                    nc.tensor.matmul(
                        ps[:],
                        xt_sb[d][:, st * 128 : (st + 1) * 128],
                        wv_sb[:, d, :],
                        start=(d == 0),
                        stop=(d == nd - 1),
                    )
                vt = v_p.tile([128, GH, 66], F16, tag="v", name=f"v{st}")
                nc.vector.memset(vt[:, 0::2, 64:65], 1.0)
                nc.vector.memset(vt[:, 1::2, 64:65], 0.0)
                nc.vector.memset(vt[:, 1::2, 65:66], 1.0)
                nc.vector.tensor_add(
                    vt[:, :, 0:64],
                    ps[:].rearrange("p (h e) -> p h e", h=GH),
                    bvb_sb[:].rearrange("p (h e) -> p h e", h=GH),
                )
                v_sb.append(vt)

            # --- per pair: Q/K projection then attention ---
            for c in range(4):
                for dst, wsb, bcol in ((qt_sb[c], wq_sb, c), (kt_sb[c], wk_sb, 4 + c)):
                    for sl in range(ns):
                        ps = ps_proj.tile([128, 512], F32, tag="ps", name="ps")
                        for d in range(nd):
                            nc.tensor.matmul(
                                ps[:],
                                wsb[:, d, c * 128 : (c + 1) * 128],
                                xt_sb[d][:, sl * 512 : (sl + 1) * 512],
                                start=(d == 0),
                                stop=(d == nd - 1),
                            )
                        nc.vector.tensor_scalar_add(
                            dst[:, sl * 512 : (sl + 1) * 512],
                            ps[:],
                            bqk_sb[:, bcol : bcol + 1],
                        )

                for j in range(ns):
                    stage = rs_p.tile([66, 512], F16, tag="rs", name="stage")
                    pv0 = ps_pv.tile([128, 512], F32, tag="pv", name="pv0")
                    pv1 = ps_pv.tile([128, 512], F32, tag="pv", name="pv1")
                    last = 4 * j + 3
                    for t in range(last + 1):
                        diag = t >= 4 * j
                        w0 = 128 * (t - 4 * j) if diag else 0
                        qk = ps_qk.tile([128, 2, 512], F32, tag="qk", name="qk")
                        for hh in range(2):
                            nc.tensor.matmul(
                                qk[:, hh, w0:512],
                                kt_sb[c][hh * 64 : hh * 64 + 64, t * 128 : (t + 1) * 128],
                                qt_sb[c][
                                    hh * 64 : hh * 64 + 64, j * 512 + w0 : (j + 1) * 512
                                ],
                                start=True,
                                stop=True,
                            )
                        et = et_p.tile([128, 2, 512], F16, tag="et", name="et")
                        nc.scalar.activation(
                            et[:, :, w0:512], qk[:, :, w0:512], AF.Exp, scale=0.125
                        )
                        if diag:
                            # zero the masked (upper) triangle of the diagonal
                            # 128x128 block with a 0/1 mask multiply
                            for hh in range(2):
                                nc.vector.tensor_mul(
                                    et[:, hh, w0 : w0 + 128],
                                    et[:, hh, w0 : w0 + 128],
                                    mask_sb[:],
                                )
                        nc.tensor.matmul(
                            pv0[0:65, w0:512],
                            v_sb[t][:, 2 * c, 0:65],
                            et[:, 0, w0:512],
                            start=(t == 0),
                            stop=(t == last),
                        )
                        nc.tensor.matmul(
                            pv1[0:66, w0:512],
                            v_sb[t][:, 2 * c + 1, 0:66],
                            et[:, 1, w0:512],
                            start=(t == 0),
                            stop=(t == last),
                        )
                    # evacuate attention outputs (unnormalized) + rowsums
                    nc.vector.tensor_copy(
                        ot_sb[c][0:64, j * 512 : (j + 1) * 512], pv0[0:64, :]
                    )
                    todd = tmp_p.tile([64, 512], F16, tag="todd", name="todd")
                    nc.vector.tensor_copy(todd[:], pv1[0:64, :])
                    nc.sync.dma_start(
                        out=ot_sb[c][64:128, j * 512 : (j + 1) * 512], in_=todd[:]
                    )
                    nc.vector.tensor_copy(stage[64:66, :], pv1[64:66, :])
                    nc.vector.tensor_copy(stage[64:65, :], pv0[64:65, :])
                    # rowsum reciprocals for this slice: scatter [2, 512]
                    # across 32 partitions, DVE-reciprocal, gather back,
                    # then replicate down all 128 partitions via DMA
                    rs128 = rs_p.tile([32, 32], F16, tag="rs128", name="rs128")
                    nc.sync.dma_start(
                        out=rs128[:],
                        in_=stage[64:66, :].rearrange("p (b e) -> p b e", e=32),
                    )
                    r128 = rs_p.tile([32, 32], F16, tag="r128", name="r128")
                    with nc.allow_low_precision(reason="softmax divisor"):
                        nc.vector.reciprocal(r128[:], rs128[:])
                    rsr = rs_p.tile([66, 512], F16, tag="rsr", name="rsr")
                    nc.sync.dma_start(
                        out=rsr[64:66, :].rearrange("p (b e) -> p b e", e=32),
                        in_=r128[:],
                    )
                    bcast = rs_p.tile([128, 512], F16, tag="bc", name="bcast")
                    rr = rsr[64:66, :]
                    nc.sync.dma_start(
                        out=bcast[:],
                        in_=bass.AP(
                            tensor=rr.tensor,
                            offset=rr.offset,
                            ap=[rr.ap[0], [0, 64], [1, 512]],
                        ),
                    )
                    nc.vector.tensor_mul(
                        ot_sb[c][:, j * 512 : (j + 1) * 512],
                        ot_sb[c][:, j * 512 : (j + 1) * 512],
                        bcast[:],
                    )
                    if c == 3:
                        # all four chunks of this j-slice are normalized:
                        # emit the output projection for its s-tiles.  For
                        # the final slice (strictly post-attention) spread
                        # the accumulators over the idle qk/pv PSUM pools so
                        # six groups can pre-accumulate chunks 0-2 while
                        # chunk 3's normalization chain completes.
                        po_pools = (
                            [(ps_proj, "ps"), (ps_qk, "qk"), (ps_pv, "pv")]
                            if j == ns - 1
                            else [(ps_proj, "ps")]
                        )
                        for sti, st in enumerate(range(4 * j, 4 * j + 4)):
                            for dsl in range(2):
                                pp, ptag = po_pools[
                                    (sti * 2 + dsl) % len(po_pools)
                                ]
                                po = pp.tile(
                                    [128, 512], F32, tag=ptag, name="po"
                                )
                                for cc in range(4):
                                    nc.tensor.matmul(
                                        po[:],
                                        ot_sb[cc][:, st * 128 : (st + 1) * 128],
                                        wo_sb[:, cc, dsl * 512 : (dsl + 1) * 512],
                                        start=(cc == 0),
                                        stop=(cc == 3),
                                    )
                                ob = ob_p.tile([128, 512], F32, tag="ob", name="ob")
                                nc.vector.tensor_copy(ob[:], po[:])
                                nc.sync.dma_start(
                                    out=out_d[
                                        st * 128 : (st + 1) * 128,
                                        dsl * 512 : (dsl + 1) * 512,
                                    ],
                                    in_=ob[:],
                                )

    if legalize:
        _legalize_waits(nc)
    return nc


_NC_CACHE = {}


def _get_nc(s=S):
    if s not in _NC_CACHE:
        _NC_CACHE[s] = build_nc(s)
    return _NC_CACHE[s]


def make_inputs(X, Wq, bq, Wk, bk, Wv, bv, Wo, bo, s=S):
    """Per-core input maps. Core c: batch c//2, head group c%2."""
    iv, jv = np.arange(128)[:, None], np.arange(128)[None, :]
    mask = (jv >= iv).astype(np.float16)
    in_maps = []
    for c in range(8):
        b, g = divmod(c, 2)
        lo, hi = g * GW, (g + 1) * GW
        bqk = np.concatenate(
            [
                np.ascontiguousarray(bq[lo:hi].reshape(4, 128).T),
                np.ascontiguousarray(bk[lo:hi].reshape(4, 128).T),
            ],
            axis=1,
        ).astype(np.float32)
        in_maps.append(
            {
                "xt": np.ascontiguousarray(X[b, :s].T).astype(np.float16),
                "wq": np.ascontiguousarray(Wq[lo:hi].T).astype(np.float16),
                "wk": np.ascontiguousarray(Wk[lo:hi].T).astype(np.float16),
                "wv": np.ascontiguousarray(Wv[lo:hi].T).astype(np.float16),
                "wo": np.ascontiguousarray(Wo[:, lo:hi].T).astype(np.float16),
                "bqk": bqk,
                "bvb": np.tile(bv[lo:hi].astype(np.float16), (128, 1)),
                "mask": mask,
            }
        )
    return in_maps


def kernel(X, Wq, bq, Wk, bk, Wv, bv, Wo, bo, **run_kwargs):
    args = [np.asarray(a, np.float32) for a in (X, Wq, bq, Wk, bk, Wv, bv, Wo, bo)]
    X, Wq, bq, Wk, bk, Wv, bv, Wo, bo = args
    nc = _get_nc(S)
    in_maps = make_inputs(X, Wq, bq, Wk, bk, Wv, bv, Wo, bo, S)
    res = run_bass_kernel_spmd(nc, in_maps, core_ids=list(range(8)), **run_kwargs)
    outs = [r["out"] for r in res.results]
    full = np.empty((B, S, D), np.float32)
    for b in range(B):
        full[b] = outs[2 * b] + outs[2 * b + 1] + bo
    kernel.last_results = res
    return full

